# revision 40
# baseline (speedup 1.0000x reference)
"""Trainium2 Bass kernel for nn_ASPMSoftmax (attention-score top-k masking).

reference:
    h = tanh(x @ W1.T + b1)            # (B,T,D)
    scores = h @ w2 + b2               # (B,T)   [b2 is a pure shift -> no-op]
    w = softmax(scores, axis=1)        # (B,T)
    mask out the int(T*0.7) lowest-scoring frames per row
    out = x * (w * mask)[..., None], (w * mask)

Sharding: data-parallel over batch, 2 samples per core on 8 cores.

Matmul precision: PE fp32 matmuls are quarter-rate and chained fp32
accumulation silently yields zeros on this toolchain, so all big matmuls use
an fp16 3-way split (a_hi*b_hi + a_hi*b_lo + a_lo*b_hi with fp32 PSUM
accumulation) which is fp32-accurate (residual ~2^-22) at full PE rate.
W1/w2 are pre-scaled by 64 host-side so their fp16 "lo" parts stay normal;
the 1/64 is folded into the ACT scale.

Top-k threshold: per sample, find t = 615th largest of 2048 scores exactly.
Scores are broadcast to all 128 partitions (K=1 ones-matmul, exact), then a
3-pass 128-ary grid search on the host-bounded range [-R, R], R = sum|w2|+1
(|score| <= sum|w2|*|tanh| < R strictly): per pass, counts of score >= t_p
for 128 grid points come from one ACT Sign(score - t_p) with accum_out
(count = (acc+T)/2), and lo' = lo + (m-1)*step via a PE step-matrix matmul
on the monotone select column. After the passes (interval 2R/128^3 ~ 2.5e-5,
plus one extra step of safety margin against fp32 update rounding),
c0 = count(score >= lo) is in [615, 622] whp, and Max8 of
(-score, masked to -2^100 below lo) yields the 8 smallest kept values
exactly; the (c0-615+1)-th of them, picked by a select chain on c0, IS the
exact 615th-largest score, so the final mask (score >= thr) keeps exactly
615 entries regardless of rounding.
"""

import numpy as np

B, T, D = 16, 2048, 1024
NCORES = 8
BPC = B // NCORES  # samples per core
NUM_MASK = int(T * 0.7)  # 1433
KEEP = T - NUM_MASK  # 615
KT = D // 128  # 8 contraction tiles
ET = D // 128  # 8 output e-tiles
TT = T // 512  # 4 tok-tiles per sample
WSCALE = 64.0
NPASS = 3  # 128-ary value passes; finer steps would sink below fp32 ulp of lo
BIG = float(2.0 ** 100)


def _split_fp16(a):
    hi = a.astype(np.float16)
    lo = (a - hi.astype(np.float32)).astype(np.float16)
    return hi, lo


def _build(R):
    import concourse.bacc as bacc
    import concourse.mybir as mybir
    import concourse.tile as tile

    f32 = mybir.dt.float32
    f16 = mybir.dt.float16
    Act = mybir.ActivationFunctionType
    Alu = mybir.AluOpType
    Ax = mybir.AxisListType

    nc = bacc.Bacc("TRN2", target_bir_lowering=False, debug=False)

    # ---- DRAM tensors ----
    d_xthi = nc.dram_tensor("xthi", [D, BPC * T], f16, kind="ExternalInput")
    d_xtlo = nc.dram_tensor("xtlo", [D, BPC * T], f16, kind="ExternalInput")
    d_x32 = nc.dram_tensor("x32", [BPC * T, D], f32, kind="ExternalInput")
    d_whi = nc.dram_tensor("whi", [D, D], f16, kind="ExternalInput")
    d_wlo = nc.dram_tensor("wlo", [D, D], f16, kind="ExternalInput")
    d_b1 = nc.dram_tensor("b1", [D], f32, kind="ExternalInput")
    d_w2hi = nc.dram_tensor("w2hi", [D], f16, kind="ExternalInput")
    d_w2lo = nc.dram_tensor("w2lo", [D], f16, kind="ExternalInput")
    d_ones = nc.dram_tensor("onesrow", [1, 128], f32, kind="ExternalInput")
    d_cgrid = nc.dram_tensor("cgrid", [NPASS, 128], f32, kind="ExternalInput")
    d_smask = nc.dram_tensor("smask", [NPASS * 128, 128], f32, kind="ExternalInput")
    d_pick = nc.dram_tensor("pick", [128, 1], f32, kind="ExternalInput")

    d_y = nc.dram_tensor("y", [BPC * T, D], f32, kind="ExternalOutput")
    d_wm = nc.dram_tensor("wm", [BPC, T], f32, kind="ExternalOutput")

    steps = [2.0 * R / (128.0 ** (k + 1)) for k in range(NPASS)]

    with tile.TileContext(nc) as tc:
        with (
            tc.tile_pool(name="res", bufs=1) as res,
            tc.tile_pool(name="xt", bufs=2) as xtp,
            tc.tile_pool(name="hp", bufs=2) as hp,
            tc.tile_pool(name="rows", bufs=2) as rows,
            tc.tile_pool(name="scr", bufs=2) as scr,
            tc.tile_pool(name="p3x", bufs=9) as p3x,
            tc.tile_pool(name="p3y", bufs=3) as p3y,
            tc.tile_pool(name="mm1ps", bufs=2, space="PSUM") as mm1ps,
            tc.tile_pool(name="mm2ps", bufs=2, space="PSUM") as mm2ps,
            tc.tile_pool(name="bigps", bufs=1, space="PSUM") as bigps,
            tc.tile_pool(name="tinyps", bufs=2, space="PSUM") as tinyps,
        ):
            # prefetch the first token tile ahead of the weights so the PE
            # can start as soon as the first k-slices land
            xh0 = xtp.tile([128, KT, 512], f16, tag="xh")
            xl0 = xtp.tile([128, KT, 512], f16, tag="xl")
            nc.sync.dma_start(
                out=xh0, in_=d_xthi[:, 0:512].rearrange("(kt p) n -> p kt n", p=128)
            )
            nc.sync.dma_start(
                out=xl0, in_=d_xtlo[:, 0:512].rearrange("(kt p) n -> p kt n", p=128)
            )

            # ---- resident weights/constants ----
            whi = res.tile([128, KT, D], f16)
            wlo = res.tile([128, KT, D], f16)
            for kt in range(KT):
                nc.sync.dma_start(
                    out=whi[:, kt, :], in_=d_whi[kt * 128 : (kt + 1) * 128, :]
                )
                nc.sync.dma_start(
                    out=wlo[:, kt, :], in_=d_wlo[kt * 128 : (kt + 1) * 128, :]
                )
            b1c = res.tile([128, ET], f32)
            nc.sync.dma_start(
                out=b1c, in_=d_b1[:].rearrange("(t p) -> p t", p=128)
            )
            w2hi = res.tile([128, ET], f16)
            w2lo = res.tile([128, ET], f16)
            nc.sync.dma_start(out=w2hi, in_=d_w2hi[:].rearrange("(t p) -> p t", p=128))
            nc.sync.dma_start(out=w2lo, in_=d_w2lo[:].rearrange("(t p) -> p t", p=128))
            onesr = res.tile([1, 128], f32)
            nc.sync.dma_start(out=onesr, in_=d_ones[:, :])
            cgrid = res.tile([128, NPASS], f32)
            nc.sync.dma_start(
                out=cgrid, in_=d_cgrid[:, :].rearrange("k p -> p k")
            )
            smask = res.tile([128, NPASS, 128], f32)
            nc.sync.dma_start(
                out=smask, in_=d_smask[:, :].rearrange("(k p) e -> p k e", p=128)
            )
            pick = res.tile([128, 1], f32)
            nc.sync.dma_start(out=pick, in_=d_pick[:, :])

            for s in range(BPC):
                # ================= scores for sample s =================
                scores = rows.tile([1, T], f32, tag="scores")
                S = scr.tile([128, T], f32, tag="S")
                for t4 in range(TT):
                    tt = s * TT + t4  # global tok tile
                    if tt == 0:
                        xh, xl = xh0, xl0
                    else:
                        xh = xtp.tile([128, KT, 512], f16, tag="xh")
                        xl = xtp.tile([128, KT, 512], f16, tag="xl")
                        nc.sync.dma_start(
                            out=xh,
                            in_=d_xthi[:, tt * 512 : (tt + 1) * 512].rearrange(
                                "(kt p) n -> p kt n", p=128
                            ),
                        )
                        nc.sync.dma_start(
                            out=xl,
                            in_=d_xtlo[:, tt * 512 : (tt + 1) * 512].rearrange(
                                "(kt p) n -> p kt n", p=128
                            ),
                        )
                    sps = mm2ps.tile([1, 512], f32, tag="sps")
                    for et in range(ET):
                        ps = mm1ps.tile([128, 512], f32, tag="ps")
                        terms = [(whi, xh), (whi, xl), (wlo, xh)]
                        n_mm = len(terms) * KT
                        i = 0
                        for kt in range(KT):
                            for wt, xt in terms:
                                nc.tensor.matmul(
                                    ps,
                                    wt[:, kt, et * 128 : (et + 1) * 128],
                                    xt[:, kt, :],
                                    start=(i == 0),
                                    stop=(i == n_mm - 1),
                                )
                                i += 1
                        h32 = hp.tile([128, 512], f32, tag="h32")
                        nc.scalar.activation(
                            out=h32,
                            in_=ps,
                            func=Act.Tanh,
                            bias=b1c[:, et : et + 1],
                            scale=1.0 / WSCALE,
                        )
                        hhi = hp.tile([128, 512], f16, tag="hhi")
                        hlo = hp.tile([128, 512], f16, tag="hlo")
                        nc.vector.tensor_copy(hhi, h32)
                        nc.vector.tensor_sub(hlo, h32, hhi)
                        sterm = [
                            (w2hi, hhi),
                            (w2hi, hlo),
                            (w2lo, hhi),
                        ]
                        for j, (wv, hv) in enumerate(sterm):
                            nc.tensor.matmul(
                                sps,
                                wv[:, et : et + 1],
                                hv,
                                start=(et == 0 and j == 0),
                                stop=(et == ET - 1 and j == len(sterm) - 1),
                            )
                    # scores chunk (undo the WSCALE on w2)
                    nc.scalar.mul(
                        scores[:, t4 * 512 : (t4 + 1) * 512], sps, 1.0 / WSCALE
                    )
                    # build the top-k score broadcast incrementally (K=1
                    # ones-matmul is exact); removes ~5us from the exposed
                    # post-mm top-k window
                    bps = bigps.tile([128, 512], f32, tag="bps")
                    nc.tensor.matmul(
                        bps,
                        onesr,
                        scores[:, t4 * 512 : (t4 + 1) * 512],
                        start=True,
                        stop=True,
                    )
                    nc.scalar.copy(S[:, t4 * 512 : (t4 + 1) * 512], bps)



                # ================= softmax =================
                m1 = rows.tile([1, 1], f32, tag="m1")
                nc.vector.tensor_reduce(out=m1, in_=scores, axis=Ax.X, op=Alu.max)
                negm = rows.tile([1, 1], f32, tag="negm")
                nc.vector.tensor_scalar_mul(negm, m1, -1.0)
                ex = rows.tile([1, T], f32, tag="ex")
                z1 = rows.tile([1, 1], f32, tag="z1")
                nc.scalar.activation(
                    out=ex, in_=scores, func=Act.Exp, bias=negm, scale=1.0,
                    accum_out=z1,
                )
                rz = rows.tile([1, 1], f32, tag="rz")
                nc.vector.reciprocal(rz, z1)
                nc.vector.tensor_scalar_mul(ex, ex, rz)  # ex <- softmax weights

                # ================= top-k threshold =================
                lo = scr.tile([128, 1], f32, tag="lo")
                nc.vector.memset(lo, -R)

                def count_acc(neg_thr_col):
                    """[128,1] acc = sum sign(s - t); count_ge = (acc+2048)/2."""
                    sgn = scr.tile([128, T], f32, tag="sgn", bufs=1)
                    acc = scr.tile([128, 1], f32, tag="acc")
                    nc.scalar.activation(
                        out=sgn, in_=S, func=Act.Sign, bias=neg_thr_col,
                        scale=1.0, accum_out=acc,
                    )
                    return acc

                for k in range(NPASS):
                    # negt = -(lo + grid_k) = -lo + ncgrid_k  (counts on ACT)
                    negt = scr.tile([128, 1], f32, tag="negt")
                    nc.vector.tensor_scalar(
                        out=negt, in0=lo, scalar1=-1.0,
                        scalar2=cgrid[:, k : k + 1], op0=Alu.mult, op1=Alu.add,
                    )
                    acc = count_acc(negt)
                    # count >= KEEP-0.5  <=>  acc >= 2*(KEEP-0.5) - T
                    selc = scr.tile([128, 1], f32, tag="selc")
                    nc.vector.tensor_scalar(
                        out=selc,
                        in0=acc,
                        scalar1=float(2 * KEEP - 1 - T),
                        scalar2=None,
                        op0=Alu.is_ge,
                    )
                    mps = tinyps.tile([128, 1], f32, tag="tiny")
                    nc.tensor.matmul(
                        mps, smask[:, k, :], selc, start=True, stop=True
                    )
                    lo2 = scr.tile([128, 1], f32, tag="lo")
                    # lo' = (lo + m*step) - step
                    tmp = scr.tile([128, 1], f32, tag="tmp")
                    nc.scalar.activation(
                        out=tmp, in_=mps, func=Act.Identity,
                        bias=lo[:, :], scale=1.0,
                    )
                    nc.vector.tensor_scalar_add(lo2, tmp, -steps[k])
                    lo = lo2
                # safety margin: one extra final-pass step down, so that
                # count(s >= lo) >= KEEP holds despite fp32 update rounding
                lom = scr.tile([128, 1], f32, tag="lo")
                nc.vector.tensor_scalar_add(lom, lo, -steps[-1])
                # c0 = count(s >= lom) in [KEEP, KEEP+7] whp; the threshold is
                # the (c0-KEEP+1)-th smallest kept value. Get the 8 smallest
                # kept at once: Max8 of (-s for kept, -BIG for masked).
                neglom = scr.tile([128, 1], f32, tag="negt")
                nc.vector.tensor_scalar_mul(neglom, lom, -1.0)
                acc0 = count_acc(neglom)
                pen = scr.tile([128, T], f32, tag="pen", bufs=1)
                nc.vector.tensor_scalar(
                    out=pen, in0=S, scalar1=lom, scalar2=-BIG,
                    op0=Alu.is_lt, op1=Alu.mult,
                )
                mneg = scr.tile([128, T], f32, tag="tmpm", bufs=1)
                nc.vector.tensor_sub(mneg, pen, S)  # kept: -s, masked: -BIG-s
                m8 = scr.tile([128, 8], f32, tag="m8")
                nc.vector.max(out=m8, in_=mneg)
                # vcol = m8[:, c0-KEEP]  (still negated)
                vcol = m8[:, 0:1]
                for j in range(1, 8):
                    # use column j iff c0 >= KEEP + j  <=> acc0 >= 2(KEEP+j)-1-T
                    aj = scr.tile([128, 1], mybir.dt.uint32, tag="actj")
                    nc.vector.tensor_scalar(
                        out=aj, in0=acc0,
                        scalar1=float(2 * (KEEP + j) - 1 - T),
                        scalar2=None, op0=Alu.is_ge,
                    )
                    vsel = scr.tile([128, 1], f32, tag="vcol")
                    nc.vector.select(vsel, aj, m8[:, j : j + 1], vcol)
                    vcol = vsel
                # thr[1,1] = -vcol[0] via pick-matmul (pick holds -1 at row 0)
                tps = tinyps.tile([1, 1], f32, tag="tiny")
                nc.tensor.matmul(tps, pick, vcol, start=True, stop=True)
                thr = rows.tile([1, 1], f32, tag="thr")
                nc.scalar.copy(thr, tps)

                # masked weights (in place): ex <- (scores >= thr) * ex
                nc.vector.scalar_tensor_tensor(
                    out=ex,
                    in0=scores,
                    scalar=thr,
                    in1=ex,
                    op0=Alu.is_ge,
                    op1=Alu.mult,
                )
                nc.sync.dma_start(out=d_wm[s : s + 1, :], in_=ex)

                # transpose wm into per-token columns: wT [128, 16]
                wps = tinyps.tile([128, 16], f32, tag="tiny")
                for blk in range(16):
                    nc.tensor.matmul(
                        wps[:, blk : blk + 1],
                        ex[:, blk * 128 : (blk + 1) * 128],
                        onesr[:, 0:1],
                        start=True,
                        stop=True,
                    )
                wT = rows.tile([128, 16], f32, tag="wT")
                nc.scalar.copy(wT, wps)

                # ================= apply: y = x * wm (on ScalarE) ========
                for blk in range(16):
                    g = s * T + blk * 128
                    xt32 = p3x.tile([128, D], f32, tag="xt32")
                    nc.sync.dma_start(out=xt32, in_=d_x32[g : g + 128, :])
                    yt = p3y.tile([128, D], f32, tag="yt")
                    nc.scalar.mul(yt, xt32, wT[:, blk : blk + 1])
                    nc.sync.dma_start(out=d_y[g : g + 128, :], in_=yt)

    nc.compile()
    return nc


def kernel(x, W1, b1, w2, b2):
    x = np.asarray(x, dtype=np.float32)
    W1 = np.asarray(W1, dtype=np.float32)
    b1 = np.asarray(b1, dtype=np.float32)
    w2 = np.asarray(w2, dtype=np.float32)

    # host-side prep
    W1T = np.ascontiguousarray(W1.T) * np.float32(WSCALE)  # [d, e]
    whi, wlo = _split_fp16(W1T)
    w2s = w2 * np.float32(WSCALE)
    w2hi, w2lo = _split_fp16(w2s)

    R = float(np.float32(np.abs(w2).sum() + 1.0))
    steps = [2.0 * R / (128.0 ** (k + 1)) for k in range(NPASS)]
    cgrid = np.zeros((NPASS, 128), dtype=np.float32)
    smask = np.zeros((NPASS, 128, 128), dtype=np.float32)
    for k in range(NPASS):
        cgrid[k, :] = -(np.arange(128, dtype=np.float32) * np.float32(steps[k]))
        smask[k, :, :] = np.float32(steps[k])
    onesr = np.ones((1, 128), dtype=np.float32)
    pick = np.zeros((128, 1), dtype=np.float32)
    pick[0, 0] = -1.0

    nc = _build(R)

    in_maps = []
    for c in range(NCORES):
        xs = x[c * BPC : (c + 1) * BPC].reshape(BPC * T, D)  # [4096, 1024]
        xT = np.ascontiguousarray(xs.T)  # [1024, 4096]
        xthi, xtlo = _split_fp16(xT)
        in_maps.append(
            dict(
                xthi=xthi,
                xtlo=xtlo,
                x32=np.ascontiguousarray(xs),
                whi=whi,
                wlo=wlo,
                b1=b1,
                w2hi=w2hi,
                w2lo=w2lo,
                onesrow=onesr,
                cgrid=cgrid,
                smask=smask.reshape(NPASS * 128, 128),
                pick=pick,
            )
        )

    from concourse.bass_utils import run_bass_kernel_spmd

    res = run_bass_kernel_spmd(nc, in_maps, core_ids=list(range(NCORES)))

    masked_output = np.empty((B, T, D), dtype=np.float32)
    masked_weights = np.empty((B, T), dtype=np.float32)
    for c in range(NCORES):
        r = res.results[c]
        masked_output[c * BPC : (c + 1) * BPC] = r["y"].reshape(BPC, T, D)
        masked_weights[c * BPC : (c + 1) * BPC] = r["wm"]
    return masked_output, masked_weights


# revision 42
# speedup vs baseline: 1.0285x; 1.0285x over previous
"""Trainium2 Bass kernel for nn_ASPMSoftmax (attention-score top-k masking).

reference:
    h = tanh(x @ W1.T + b1)            # (B,T,D)
    scores = h @ w2 + b2               # (B,T)   [b2 is a pure shift -> no-op]
    w = softmax(scores, axis=1)        # (B,T)
    mask out the int(T*0.7) lowest-scoring frames per row
    out = x * (w * mask)[..., None], (w * mask)

Sharding: data-parallel over batch, 2 samples per core on 8 cores.

Matmul precision: PE fp32 matmuls are quarter-rate and chained fp32
accumulation silently yields zeros on this toolchain, so all big matmuls use
an fp16 3-way split (a_hi*b_hi + a_hi*b_lo + a_lo*b_hi with fp32 PSUM
accumulation) which is fp32-accurate (residual ~2^-22) at full PE rate.
W1/w2 are pre-scaled by 64 host-side so their fp16 "lo" parts stay normal;
the 1/64 is folded into the ACT scale.

Top-k threshold: per sample, find t = 615th largest of 2048 scores exactly.
Scores are broadcast to all 128 partitions (K=1 ones-matmul, exact), then a
3-pass 128-ary grid search on the host-bounded range [-R, R], R = sum|w2|+1
(|score| <= sum|w2|*|tanh| < R strictly): per pass, counts of score >= t_p
for 128 grid points come from one ACT Sign(score - t_p) with accum_out
(count = (acc+T)/2), and lo' = lo + (m-1)*step via a PE step-matrix matmul
on the monotone select column. After the passes (interval 2R/128^3 ~ 2.5e-5,
plus one extra step of safety margin against fp32 update rounding),
c0 = count(score >= lo) is in [615, 622] whp, and Max8 of
(-score, masked to -2^100 below lo) yields the 8 smallest kept values
exactly; the (c0-615+1)-th of them, picked by a select chain on c0, IS the
exact 615th-largest score, so the final mask (score >= thr) keeps exactly
615 entries regardless of rounding.
"""

import numpy as np

B, T, D = 16, 2048, 1024
NCORES = 8
BPC = B // NCORES  # samples per core
NUM_MASK = int(T * 0.7)  # 1433
KEEP = T - NUM_MASK  # 615
KT = D // 128  # 8 contraction tiles
ET = D // 128  # 8 output e-tiles
TT = T // 512  # 4 tok-tiles per sample
WSCALE = 64.0
NPASS = 3  # 128-ary value passes; finer steps would sink below fp32 ulp of lo
BIG = float(2.0 ** 100)


def _split_fp16(a):
    hi = a.astype(np.float16)
    lo = (a - hi.astype(np.float32)).astype(np.float16)
    return hi, lo


def _build(R):
    import concourse.bacc as bacc
    import concourse.mybir as mybir
    import concourse.tile as tile

    f32 = mybir.dt.float32
    f16 = mybir.dt.float16
    Act = mybir.ActivationFunctionType
    Alu = mybir.AluOpType
    Ax = mybir.AxisListType

    nc = bacc.Bacc("TRN2", target_bir_lowering=False, debug=False)

    # ---- DRAM tensors ----
    d_xthi = nc.dram_tensor("xthi", [D, BPC * T], f16, kind="ExternalInput")
    d_xtlo = nc.dram_tensor("xtlo", [D, BPC * T], f16, kind="ExternalInput")
    d_x32 = nc.dram_tensor("x32", [BPC * T, D], f32, kind="ExternalInput")
    d_whi = nc.dram_tensor("whi", [D, D], f16, kind="ExternalInput")
    d_wlo = nc.dram_tensor("wlo", [D, D], f16, kind="ExternalInput")
    d_b1 = nc.dram_tensor("b1", [D], f32, kind="ExternalInput")
    d_w2p = nc.dram_tensor("w2pair", [D, 2], f16, kind="ExternalInput")
    d_ones = nc.dram_tensor("onesrow", [2, 128], f32, kind="ExternalInput")
    d_cgrid = nc.dram_tensor("cgrid", [NPASS, 128], f32, kind="ExternalInput")
    d_smask = nc.dram_tensor("smask", [NPASS * 128, 128], f32, kind="ExternalInput")
    d_pick = nc.dram_tensor("pick", [128, 1], f32, kind="ExternalInput")

    d_y = nc.dram_tensor("y", [BPC * T, D], f32, kind="ExternalOutput")
    d_wm = nc.dram_tensor("wm", [BPC, T], f32, kind="ExternalOutput")

    steps = [2.0 * R / (128.0 ** (k + 1)) for k in range(NPASS)]

    with tile.TileContext(nc) as tc:
        with (
            tc.tile_pool(name="res", bufs=1) as res,
            tc.tile_pool(name="xt", bufs=2) as xtp,
            tc.tile_pool(name="hp", bufs=2) as hp,
            tc.tile_pool(name="rows", bufs=2) as rows,
            tc.tile_pool(name="scr", bufs=2) as scr,
            tc.tile_pool(name="p3x", bufs=9) as p3x,
            tc.tile_pool(name="p3y", bufs=3) as p3y,
            tc.tile_pool(name="mm1ps", bufs=2, space="PSUM") as mm1ps,
            tc.tile_pool(name="mm2ps", bufs=2, space="PSUM") as mm2ps,
            tc.tile_pool(name="bigps", bufs=1, space="PSUM") as bigps,
            tc.tile_pool(name="tinyps", bufs=2, space="PSUM") as tinyps,
        ):
            # prefetch the first token tile ahead of the weights so the PE
            # can start as soon as the first k-slices land
            xh0 = xtp.tile([128, KT, 512], f16, tag="xh")
            xl0 = xtp.tile([128, KT, 512], f16, tag="xl")
            nc.sync.dma_start(
                out=xh0, in_=d_xthi[:, 0:512].rearrange("(kt p) n -> p kt n", p=128)
            )
            nc.sync.dma_start(
                out=xl0, in_=d_xtlo[:, 0:512].rearrange("(kt p) n -> p kt n", p=128)
            )

            # ---- resident weights/constants ----
            whi = res.tile([128, KT, D], f16)
            wlo = res.tile([128, KT, D], f16)
            for kt in range(KT):
                nc.sync.dma_start(
                    out=whi[:, kt, :], in_=d_whi[kt * 128 : (kt + 1) * 128, :]
                )
                nc.sync.dma_start(
                    out=wlo[:, kt, :], in_=d_wlo[kt * 128 : (kt + 1) * 128, :]
                )
            b1c = res.tile([128, ET], f32)
            nc.sync.dma_start(
                out=b1c, in_=d_b1[:].rearrange("(t p) -> p t", p=128)
            )
            w2p = res.tile([128, ET, 2], f16)
            nc.sync.dma_start(
                out=w2p, in_=d_w2p[:, :].rearrange("(t p) c -> p t c", p=128)
            )
            onesr = res.tile([2, 128], f32)
            nc.sync.dma_start(out=onesr, in_=d_ones[:, :])
            cgrid = res.tile([128, NPASS], f32)
            nc.sync.dma_start(
                out=cgrid, in_=d_cgrid[:, :].rearrange("k p -> p k")
            )
            smask = res.tile([128, NPASS, 128], f32)
            nc.sync.dma_start(
                out=smask, in_=d_smask[:, :].rearrange("(k p) e -> p k e", p=128)
            )
            pick = res.tile([128, 1], f32)
            nc.sync.dma_start(out=pick, in_=d_pick[:, :])

            for s in range(BPC):
                # ================= scores for sample s =================
                scores2 = rows.tile([2, T], f32, tag="scores")
                for t4 in range(TT):
                    tt = s * TT + t4  # global tok tile
                    if tt == 0:
                        xh, xl = xh0, xl0
                    else:
                        xh = xtp.tile([128, KT, 512], f16, tag="xh")
                        xl = xtp.tile([128, KT, 512], f16, tag="xl")
                        nc.sync.dma_start(
                            out=xh,
                            in_=d_xthi[:, tt * 512 : (tt + 1) * 512].rearrange(
                                "(kt p) n -> p kt n", p=128
                            ),
                        )
                        nc.sync.dma_start(
                            out=xl,
                            in_=d_xtlo[:, tt * 512 : (tt + 1) * 512].rearrange(
                                "(kt p) n -> p kt n", p=128
                            ),
                        )
                    sps = mm2ps.tile([2, 512], f32, tag="sps")
                    for et in range(ET):
                        ps = mm1ps.tile([128, 512], f32, tag="ps")
                        terms = [(whi, xh), (whi, xl), (wlo, xh)]
                        n_mm = len(terms) * KT
                        i = 0
                        for kt in range(KT):
                            for wt, xt in terms:
                                nc.tensor.matmul(
                                    ps,
                                    wt[:, kt, et * 128 : (et + 1) * 128],
                                    xt[:, kt, :],
                                    start=(i == 0),
                                    stop=(i == n_mm - 1),
                                )
                                i += 1
                        h32 = hp.tile([128, 512], f32, tag="h32")
                        nc.scalar.activation(
                            out=h32,
                            in_=ps,
                            func=Act.Tanh,
                            bias=b1c[:, et : et + 1],
                            scale=1.0 / WSCALE,
                        )
                        hhi = hp.tile([128, 512], f16, tag="hhi")
                        hlo = hp.tile([128, 512], f16, tag="hlo")
                        nc.vector.tensor_copy(hhi, h32)
                        nc.vector.tensor_sub(hlo, h32, hhi)
                        # rows: 0 += w2hi.hhi + w2hi.hlo ; 1 += w2lo.hhi
                        nc.tensor.matmul(
                            sps,
                            w2p[:, et, :],
                            hhi,
                            start=(et == 0),
                            stop=False,
                        )
                        nc.tensor.matmul(
                            sps[0:1, :],
                            w2p[:, et, 0:1],
                            hlo,
                            start=False,
                            stop=(et == ET - 1),
                        )
                    # scores chunk rows (undo the WSCALE on w2)
                    nc.scalar.mul(
                        scores2[:, t4 * 512 : (t4 + 1) * 512], sps, 1.0 / WSCALE
                    )



                # combined scores: S[p,:] = scores2[0,:]+scores2[1,:] (K=2
                # ones-matmul, one fp32 rounding, consistent everywhere)
                bps = bigps.tile([128, 1024], f32, tag="bps")
                S = scr.tile([128, T], f32, tag="S")
                for c in range(2):
                    for cc in range(2):
                        nc.tensor.matmul(
                            bps[:, cc * 512 : (cc + 1) * 512],
                            onesr,
                            scores2[:, (c * 2 + cc) * 512 : (c * 2 + cc + 1) * 512],
                            start=True,
                            stop=True,
                        )
                    nc.scalar.copy(S[:, c * 1024 : (c + 1) * 1024], bps)
                scores = S[0:1, :]

                # ================= softmax =================
                m1 = rows.tile([1, 1], f32, tag="m1")
                nc.vector.tensor_reduce(out=m1, in_=scores, axis=Ax.X, op=Alu.max)
                negm = rows.tile([1, 1], f32, tag="negm")
                nc.vector.tensor_scalar_mul(negm, m1, -1.0)
                ex = rows.tile([1, T], f32, tag="ex")
                z1 = rows.tile([1, 1], f32, tag="z1")
                nc.scalar.activation(
                    out=ex, in_=scores, func=Act.Exp, bias=negm, scale=1.0,
                    accum_out=z1,
                )
                rz = rows.tile([1, 1], f32, tag="rz")
                nc.vector.reciprocal(rz, z1)
                nc.vector.tensor_scalar_mul(ex, ex, rz)  # ex <- softmax weights

                # ================= top-k threshold =================
                lo = scr.tile([128, 1], f32, tag="lo")
                nc.vector.memset(lo, -R)

                def count_acc(neg_thr_col):
                    """[128,1] acc = sum sign(s - t); count_ge = (acc+2048)/2."""
                    sgn = scr.tile([128, T], f32, tag="sgn", bufs=1)
                    acc = scr.tile([128, 1], f32, tag="acc")
                    nc.scalar.activation(
                        out=sgn, in_=S, func=Act.Sign, bias=neg_thr_col,
                        scale=1.0, accum_out=acc,
                    )
                    return acc

                for k in range(NPASS):
                    # negt = -(lo + grid_k) = -lo + ncgrid_k  (counts on ACT)
                    negt = scr.tile([128, 1], f32, tag="negt")
                    nc.vector.tensor_scalar(
                        out=negt, in0=lo, scalar1=-1.0,
                        scalar2=cgrid[:, k : k + 1], op0=Alu.mult, op1=Alu.add,
                    )
                    acc = count_acc(negt)
                    # count >= KEEP-0.5  <=>  acc >= 2*(KEEP-0.5) - T
                    selc = scr.tile([128, 1], f32, tag="selc")
                    nc.vector.tensor_scalar(
                        out=selc,
                        in0=acc,
                        scalar1=float(2 * KEEP - 1 - T),
                        scalar2=None,
                        op0=Alu.is_ge,
                    )
                    mps = tinyps.tile([128, 1], f32, tag="tiny")
                    nc.tensor.matmul(
                        mps, smask[:, k, :], selc, start=True, stop=True
                    )
                    lo2 = scr.tile([128, 1], f32, tag="lo")
                    # lo' = (lo + m*step) - step
                    tmp = scr.tile([128, 1], f32, tag="tmp")
                    nc.scalar.activation(
                        out=tmp, in_=mps, func=Act.Identity,
                        bias=lo[:, :], scale=1.0,
                    )
                    nc.vector.tensor_scalar_add(lo2, tmp, -steps[k])
                    lo = lo2
                # safety margin: one extra final-pass step down, so that
                # count(s >= lo) >= KEEP holds despite fp32 update rounding
                lom = scr.tile([128, 1], f32, tag="lo")
                nc.vector.tensor_scalar_add(lom, lo, -steps[-1])
                # c0 = count(s >= lom) in [KEEP, KEEP+7] whp; the threshold is
                # the (c0-KEEP+1)-th smallest kept value. Get the 8 smallest
                # kept at once: Max8 of (-s for kept, -BIG for masked).
                neglom = scr.tile([128, 1], f32, tag="negt")
                nc.vector.tensor_scalar_mul(neglom, lom, -1.0)
                acc0 = count_acc(neglom)
                pen = scr.tile([128, T], f32, tag="pen", bufs=1)
                nc.vector.tensor_scalar(
                    out=pen, in0=S, scalar1=lom, scalar2=-BIG,
                    op0=Alu.is_lt, op1=Alu.mult,
                )
                mneg = scr.tile([128, T], f32, tag="tmpm", bufs=1)
                nc.vector.tensor_sub(mneg, pen, S)  # kept: -s, masked: -BIG-s
                m8 = scr.tile([128, 8], f32, tag="m8")
                nc.vector.max(out=m8, in_=mneg)
                # vcol = m8[:, c0-KEEP]  (still negated)
                vcol = m8[:, 0:1]
                for j in range(1, 8):
                    # use column j iff c0 >= KEEP + j  <=> acc0 >= 2(KEEP+j)-1-T
                    aj = scr.tile([128, 1], mybir.dt.uint32, tag="actj")
                    nc.vector.tensor_scalar(
                        out=aj, in0=acc0,
                        scalar1=float(2 * (KEEP + j) - 1 - T),
                        scalar2=None, op0=Alu.is_ge,
                    )
                    vsel = scr.tile([128, 1], f32, tag="vcol")
                    nc.vector.select(vsel, aj, m8[:, j : j + 1], vcol)
                    vcol = vsel
                # thr[1,1] = -vcol[0] via pick-matmul (pick holds -1 at row 0)
                tps = tinyps.tile([1, 1], f32, tag="tiny")
                nc.tensor.matmul(tps, pick, vcol, start=True, stop=True)
                thr = rows.tile([1, 1], f32, tag="thr")
                nc.scalar.copy(thr, tps)

                # masked weights (in place): ex <- (scores >= thr) * ex
                nc.vector.scalar_tensor_tensor(
                    out=ex,
                    in0=scores,
                    scalar=thr,
                    in1=ex,
                    op0=Alu.is_ge,
                    op1=Alu.mult,
                )
                nc.sync.dma_start(out=d_wm[s : s + 1, :], in_=ex)

                # transpose wm into per-token columns: wT [128, 16]
                wps = tinyps.tile([128, 16], f32, tag="tiny")
                for blk in range(16):
                    nc.tensor.matmul(
                        wps[:, blk : blk + 1],
                        ex[:, blk * 128 : (blk + 1) * 128],
                        onesr[0:1, 0:1],
                        start=True,
                        stop=True,
                    )
                wT = rows.tile([128, 16], f32, tag="wT")
                nc.scalar.copy(wT, wps)

                # ================= apply: y = x * wm (on ScalarE) ========
                for blk in range(16):
                    g = s * T + blk * 128
                    xt32 = p3x.tile([128, D], f32, tag="xt32")
                    nc.sync.dma_start(out=xt32, in_=d_x32[g : g + 128, :])
                    yt = p3y.tile([128, D], f32, tag="yt")
                    nc.scalar.mul(yt, xt32, wT[:, blk : blk + 1])
                    nc.sync.dma_start(out=d_y[g : g + 128, :], in_=yt)

    nc.compile()
    return nc


def kernel(x, W1, b1, w2, b2):
    x = np.asarray(x, dtype=np.float32)
    W1 = np.asarray(W1, dtype=np.float32)
    b1 = np.asarray(b1, dtype=np.float32)
    w2 = np.asarray(w2, dtype=np.float32)

    # host-side prep
    W1T = np.ascontiguousarray(W1.T) * np.float32(WSCALE)  # [d, e]
    whi, wlo = _split_fp16(W1T)
    w2s = w2 * np.float32(WSCALE)
    w2hi, w2lo = _split_fp16(w2s)
    w2pair = np.ascontiguousarray(np.stack([w2hi, w2lo], axis=1))

    R = float(np.float32(np.abs(w2).sum() + 1.0))
    steps = [2.0 * R / (128.0 ** (k + 1)) for k in range(NPASS)]
    cgrid = np.zeros((NPASS, 128), dtype=np.float32)
    smask = np.zeros((NPASS, 128, 128), dtype=np.float32)
    for k in range(NPASS):
        cgrid[k, :] = -(np.arange(128, dtype=np.float32) * np.float32(steps[k]))
        smask[k, :, :] = np.float32(steps[k])
    onesr = np.ones((2, 128), dtype=np.float32)
    pick = np.zeros((128, 1), dtype=np.float32)
    pick[0, 0] = -1.0

    nc = _build(R)

    in_maps = []
    for c in range(NCORES):
        xs = x[c * BPC : (c + 1) * BPC].reshape(BPC * T, D)  # [4096, 1024]
        xT = np.ascontiguousarray(xs.T)  # [1024, 4096]
        xthi, xtlo = _split_fp16(xT)
        in_maps.append(
            dict(
                xthi=xthi,
                xtlo=xtlo,
                x32=np.ascontiguousarray(xs),
                whi=whi,
                wlo=wlo,
                b1=b1,
                w2pair=w2pair,
                onesrow=onesr,
                cgrid=cgrid,
                smask=smask.reshape(NPASS * 128, 128),
                pick=pick,
            )
        )

    from concourse.bass_utils import run_bass_kernel_spmd

    res = run_bass_kernel_spmd(nc, in_maps, core_ids=list(range(NCORES)))

    masked_output = np.empty((B, T, D), dtype=np.float32)
    masked_weights = np.empty((B, T), dtype=np.float32)
    for c in range(NCORES):
        r = res.results[c]
        masked_output[c * BPC : (c + 1) * BPC] = r["y"].reshape(BPC, T, D)
        masked_weights[c * BPC : (c + 1) * BPC] = r["wm"]
    return masked_output, masked_weights
